# revision 9
# baseline (speedup 1.0000x reference)
"""Trainium2 Bass kernel for nn_Conv2d_86191403696259.

Conv2d: x [3, 2048, 2048] f32, weight [16, 3, 3, 3], stride 1, pad 1
-> out [16, 2048, 2048] f32.

Strategy (8 NeuronCores, SPMD over H):
  - Each core computes a 256-row horizontal slab of the output.
  - Host pads x to [3, 2050, 2050] (zero pad=1) and hands core c the slab
    x_pad[:, 256c : 256c+258, :]  (258 rows = 256 + 2 halo).
  - In-SBUF layout ("bands"): partition p = 24*dh + 3*j + ic  (72 partitions)
    holds, for band j (output rows 32j..32j+31 of the slab), channel ic,
    the input rows shifted by dh.  With block-diagonal weights
    lhsT[24dh+3j+ic, 16j+oc] = W[oc, ic, dh, dw], a single matmul
    [K=72] x [M=128] computes 8 output rows (one per band) x 512 px.
    The kw (dw) dimension is 3 accumulating matmuls with free-dim offsets.
  - fp32r matmuls (1 cycle/row for N=512).
  - PSUM [128, 2048] (4 banks / s-step) -> DVE copy -> SBUF stage -> DMA out.
"""

import numpy as np

import concourse.bass as bass
import concourse.mybir as mybir
import concourse.tile as tile
from concourse import bacc
from concourse.bass_utils import run_bass_kernel_spmd

# problem constants
IC, OC, KH, KW = 3, 16, 3, 3
H = W = 2048
N_CORES = 8
RPC = H // N_CORES          # rows per core = 256
HP = RPC + 2                # padded slab rows per core = 258
WP = W + 2                  # padded width = 2050

NB = 8                      # bands per core
BR = RPC // NB              # rows per band = 32
S = 4                       # s-steps per chunk
NCHUNK = BR // S            # 8 chunks
NWT = W // 512              # 4 w-tiles of 512

F32 = mybir.dt.float32
F32R = mybir.dt.float32r


def build_nc() -> bass.Bass:
    nc = bacc.Bacc("TRN2", target_bir_lowering=False, debug=False)
    x = nc.dram_tensor("x", [IC, HP, WP], F32R, kind="ExternalInput")
    wd = nc.dram_tensor("wd", [KW, 72, 128], F32R, kind="ExternalInput")
    out = nc.dram_tensor("out", [OC, RPC, W], F32, kind="ExternalOutput")

    with tile.TileContext(nc) as tc:
        with (
            tc.tile_pool(name="wpool", bufs=1) as wpool,
            tc.tile_pool(name="slab", bufs=2) as slab_pool,
            tc.tile_pool(name="stage", bufs=2) as stage_pool,
            tc.tile_pool(name="psum", bufs=2, space="PSUM") as psum_pool,
        ):
            w_sb = wpool.tile([72, KW * 128], F32R)
            # wd[dw, p, m] -> w_sb[p, dw*128 + m]
            nc.sync.dma_start(
                out=w_sb[:, :],
                in_=wd.rearrange("dw p m -> p dw m"),
            )

            for kc in range(NCHUNK):
                slab = slab_pool.tile([72, S * WP], F32R, tag="slab")
                # load: partition 24*dh + 3*j + ic, free (s, w) holds
                # x[ic, 32j + S*kc + s + dh, w]
                for dh in range(KH):
                    for s in range(S):
                        rs = S * kc + s + dh
                        src = x[:, rs : rs + (NB - 1) * BR + 1 : BR, :]
                        nc.sync.dma_start(
                            out=slab[24 * dh : 24 * dh + 24, s * WP : (s + 1) * WP],
                            in_=src.rearrange("ic j w -> j ic w"),
                        )

                stg = stage_pool.tile([128, S * W], F32, tag="stage")
                for s in range(S):
                    ps = psum_pool.tile([128, W], F32, tag="ps")
                    for dw in range(KW):
                        for wt in range(NWT):
                            nc.tensor.matmul(
                                out=ps[:, wt * 512 : (wt + 1) * 512],
                                lhsT=w_sb[:, dw * 128 : (dw + 1) * 128],
                                rhs=slab[
                                    :, s * WP + wt * 512 + dw : s * WP + wt * 512 + dw + 512
                                ],
                                start=(dw == 0),
                                stop=(dw == KW - 1),
                            )
                    nc.vector.tensor_copy(out=stg[:, s * W : (s + 1) * W], in_=ps[:, :])

                # store: partition 16j+oc, free (s, w) -> out[oc, 32j + S*kc + s, w]
                for s in range(S):
                    rs = S * kc + s
                    dst = out[:, rs : rs + (NB - 1) * BR + 1 : BR, :]
                    nc.sync.dma_start(
                        out=dst.rearrange("oc j w -> j oc w"),
                        in_=stg[:, s * W : (s + 1) * W],
                    )

    nc.compile()
    return nc


def make_wdiag(kernel: np.ndarray) -> np.ndarray:
    """kernel [OC, IC, KH, KW] -> block-diag lhsT stack [KW, 72, 128]."""
    wd = np.zeros((KW, 72, 128), np.float32)
    for dw in range(KW):
        for dh in range(KH):
            for j in range(NB):
                for ic in range(IC):
                    wd[dw, 24 * dh + 3 * j + ic, 16 * j : 16 * j + OC] = kernel[
                        :, ic, dh, dw
                    ]
    return wd


_NC_CACHE = {}


def kernel(x: np.ndarray, kernel: np.ndarray) -> np.ndarray:
    assert x.shape == (IC, H, W) and kernel.shape == (OC, IC, KH, KW)
    x = np.ascontiguousarray(x, np.float32)
    kernel = np.ascontiguousarray(kernel, np.float32)

    if "nc" not in _NC_CACHE:
        _NC_CACHE["nc"] = build_nc()
    nc = _NC_CACHE["nc"]

    x_pad = np.zeros((IC, H + 2, W + 2), np.float32)
    x_pad[:, 1:-1, 1:-1] = x
    wd = make_wdiag(kernel)

    in_maps = []
    for c in range(N_CORES):
        slab = np.ascontiguousarray(x_pad[:, c * RPC : c * RPC + HP, :])
        in_maps.append({"x": slab, "wd": wd})

    res = run_bass_kernel_spmd(nc, in_maps, core_ids=list(range(N_CORES)))
    outs = [res.results[c]["out"] for c in range(N_CORES)]
    return np.concatenate(outs, axis=1)
